# revision 4
# baseline (speedup 1.0000x reference)
"""Trainium2 Bass kernel for MQA attention (B=4, T=1024, D=2048, 16 q-heads, 1 kv-head).

Sharding: 8 cores = 4 batches x 2 head-groups (8 query heads each).
Each core computes, for its batch b and head-group g:
  - x^T via PE transposes (contraction layouts need D on partitions)
  - k/v projections (shared single KV head, duplicated across the pair)
  - RoPE on q/k in [H, tok] layout using host-precomputed sin/cos tables
  - causal attention in transposed-logits layout (logits^T = [k, q]) so that
    PV needs no transposes; softmax denominator rides as a fused ones-column
    of the PV rhs; no max-subtraction (logits are bounded by construction)
  - output projection for its 8 heads -> partial [T, D]
Host sums the two partials per batch (the pair all-reduce) and stacks batches.

The SPMD program is identical on all cores; only the data differs.
"""

import numpy as np
import concourse.bass as bass
import concourse.mybir as mybir
from concourse import bacc
from concourse.tile import TileContext
from concourse.bass_utils import run_bass_kernel_spmd
from concourse.masks import make_identity
from contextlib import ExitStack

F32 = mybir.dt.float32
# Matmul compute dtype: float32 (exact), float32r (fast fp32, 4x at N>=256).
MM_DT = F32

B, T, D, NH, HD = 4, 1024, 2048, 16, 128
HHD = HD // 2          # 64, rope half
NL = NH // 2           # 8 heads per core
DC = D // 128          # 16 contraction chunks
TT = T // 128          # 8 token tiles
EXPAD = 256            # PV rhs width: [v (128) | ones (1) | zeros (127)]

# Rope-pair interleave: the H dim of q/k is permuted (consistently in wq/wk
# columns, host-side) so each rope pair (f, f+64) sits 16 lanes apart within
# one 32-partition quadrant; the rotate-half becomes a stream_shuffle.
SHUF_MASK = list(range(16, 32)) + list(range(16))


def _mm(ap):
    return ap if MM_DT == F32 else ap.bitcast(MM_DT)


def _rope(nc, out, pin, cos, sin, tmp):
    """RoPE in permuted [H, tok] layout. pin: [128, W] (psum), cos: duplicated
    cos table, sin: sign-baked sin table (-sin on first-half lanes, +sin on
    second-half lanes), tmp: [128, W] sbuf scratch.
    out = pin * cos + shuffle16(pin) * sin.
    """
    nc.vector.stream_shuffle(tmp, pin, SHUF_MASK)
    nc.vector.tensor_mul(out, pin, cos)
    nc.vector.tensor_mul(tmp, tmp, sin)
    nc.vector.tensor_add(out, out, tmp)


def build_nc():
    nc = bacc.Bacc("TRN2", target_bir_lowering=False, debug=False, num_devices=8)
    dt = F32
    x_d = nc.dram_tensor("x", [T, D], dt, kind="ExternalInput").ap()
    wq_d = nc.dram_tensor("wq", [NL, D, HD], dt, kind="ExternalInput").ap()
    wk_d = nc.dram_tensor("wk", [D, HD], dt, kind="ExternalInput").ap()
    wv_d = nc.dram_tensor("wv", [D, HD], dt, kind="ExternalInput").ap()
    wo_d = nc.dram_tensor("wo", [NL, HD, D], dt, kind="ExternalInput").ap()
    cosq_d = nc.dram_tensor("cosq", [128, T], dt, kind="ExternalInput").ap()
    sinq_d = nc.dram_tensor("sinq", [128, T], dt, kind="ExternalInput").ap()
    cosk_d = nc.dram_tensor("cosk", [128, T], dt, kind="ExternalInput").ap()
    sink_d = nc.dram_tensor("sink", [128, T], dt, kind="ExternalInput").ap()
    tri_d = nc.dram_tensor("tri", [128, 128], dt, kind="ExternalInput").ap()
    out_d = nc.dram_tensor("out", [T, D], dt, kind="ExternalOutput").ap()

    with TileContext(nc) as tc, ExitStack() as ctx:
        singles = ctx.enter_context(tc.tile_pool(name="singles", bufs=1))
        ident = singles.tile([128, 128], dt)
        make_identity(nc, ident)
        tri = singles.tile([128, 128], dt)
        nc.sync.dma_start(out=tri, in_=tri_d)
        cosq = singles.tile([128, T], dt)
        sinq = singles.tile([128, T], dt)
        cosk = singles.tile([128, T], dt)
        sink = singles.tile([128, T], dt)
        nc.sync.dma_start(out=cosq, in_=cosq_d)
        nc.sync.dma_start(out=sinq, in_=sinq_d)
        nc.sync.dma_start(out=cosk, in_=cosk_d)
        nc.sync.dma_start(out=sink, in_=sink_d)
        wk_sb = singles.tile([128, DC, HD], dt)
        wv_sb = singles.tile([128, DC, HD], dt)
        nc.sync.dma_start(out=wk_sb, in_=wk_d.rearrange("(c p) h -> p c h", p=128))
        nc.sync.dma_start(out=wv_sb, in_=wv_d.rearrange("(c p) h -> p c h", p=128))
        xT = singles.tile([128, DC, T], dt)        # x transposed, 8MB
        kT = singles.tile([128, T], dt)            # roped k^T
        vext = singles.tile([128, TT, EXPAD], dt)  # v | ones | zeros
        encT = singles.tile([128, NL, TT, 128], dt)  # encoded^T per head, 4MB

        # ---- phase 0: load x, build xT ----
        with tc.tile_pool(name="xload", bufs=2) as xload, \
             tc.tile_pool(name="ptr", bufs=2, space="PSUM") as ptr:
            for t in range(TT):
                xn = xload.tile([128, D], dt)
                nc.sync.dma_start(out=xn, in_=x_d[t * 128:(t + 1) * 128, :])
                for c in range(DC):
                    pt = ptr.tile([128, 128], dt)
                    nc.tensor.transpose(pt, xn[:, c * 128:(c + 1) * 128], ident)
                    nc.scalar.copy(out=xT[:, c, t * 128:(t + 1) * 128], in_=pt)

        # ---- phase 1: k^T (roped) and v_ext ----
        with tc.tile_pool(name="pk1", bufs=2, space="PSUM") as pk1, \
             tc.tile_pool(name="pv1", bufs=2, space="PSUM") as pv1, \
             tc.tile_pool(name="ktmp", bufs=2) as ktmp:
            for th in range(2):
                sl = slice(th * 512, (th + 1) * 512)
                pk = pk1.tile([128, 512], dt)
                for c in range(DC):
                    nc.tensor.matmul(pk, _mm(wk_sb[:, c, :]), _mm(xT[:, c, sl]),
                                     start=(c == 0), stop=(c == DC - 1))
                tmp = ktmp.tile([128, 512], dt)
                _rope(nc, kT[:, sl], pk[:, :], cosk[:, sl], sink[:, sl], tmp)
            for tb in range(TT):
                pv = pv1.tile([128, 128], dt)
                for c in range(DC):
                    nc.tensor.matmul(pv, _mm(xT[:, c, tb * 128:(tb + 1) * 128]),
                                     _mm(wv_sb[:, c, :]),
                                     start=(c == 0), stop=(c == DC - 1))
                nc.scalar.copy(out=vext[:, tb, 0:128], in_=pv)
                nc.vector.memset(vext[:, tb, 128:129], 1.0)
                nc.vector.memset(vext[:, tb, 129:EXPAD], 0.0)

        # ---- phase 2: per-head q-proj + rope + causal attention ----
        with tc.tile_pool(name="wqp", bufs=2) as wqp, \
             tc.tile_pool(name="qtp", bufs=2) as qtp, \
             tc.tile_pool(name="ropet", bufs=2) as ropet, \
             tc.tile_pool(name="expp", bufs=3) as expp, \
             tc.tile_pool(name="encp", bufs=2) as encp, \
             tc.tile_pool(name="recp", bufs=2) as recp, \
             tc.tile_pool(name="pq2", bufs=2, space="PSUM") as pq2, \
             tc.tile_pool(name="pl2", bufs=2, space="PSUM") as pl2, \
             tc.tile_pool(name="pe2", bufs=1, space="PSUM") as pe2, \
             tc.tile_pool(name="pt2", bufs=2, space="PSUM") as pt2:
            for n in range(NL):
                wq_sb = wqp.tile([128, DC, HD], dt)
                nc.sync.dma_start(out=wq_sb,
                                  in_=wq_d[n].rearrange("(c p) h -> p c h", p=128))
                qT = qtp.tile([128, T], dt)
                for th in range(2):
                    sl = slice(th * 512, (th + 1) * 512)
                    pq = pq2.tile([128, 512], dt)
                    for c in range(DC):
                        nc.tensor.matmul(pq, _mm(wq_sb[:, c, :]), _mm(xT[:, c, sl]),
                                         start=(c == 0), stop=(c == DC - 1))
                    tmp = ropet.tile([128, 512], dt)
                    _rope(nc, qT[:, sl], pq, cosq[:, sl], sinq[:, sl], tmp)
                for qb in range(4):          # q blocks of 256 rows
                    R = qb * 256
                    d0 = R // 128            # diagonal chunk of sub0
                    d1 = d0 + 1              # diagonal chunk of sub1 (last)
                    pe0 = pe2.tile([128, EXPAD], dt, tag="pe0", name="pe0")
                    pe1 = pe2.tile([128, EXPAD], dt, tag="pe1", name="pe1")
                    for kc in range(d1 + 1):
                        plt = pl2.tile([128, 256], dt)
                        nc.tensor.matmul(plt, _mm(kT[:, kc * 128:(kc + 1) * 128]),
                                         _mm(qT[:, R:R + 256]),
                                         start=True, stop=True)
                        ex = expp.tile([128, 256], dt)
                        nc.scalar.activation(out=ex, in_=plt,
                                             func=mybir.ActivationFunctionType.Exp)
                        if kc == d0:
                            nc.vector.tensor_mul(ex[:, 0:128], ex[:, 0:128], tri)
                        if kc == d1:
                            nc.vector.tensor_mul(ex[:, 128:256], ex[:, 128:256], tri)
                        if kc <= d0:
                            nc.tensor.matmul(pe0, _mm(ex[:, 0:128]),
                                             _mm(vext[:, kc, :]),
                                             start=(kc == 0), stop=(kc == d0))
                        nc.tensor.matmul(pe1, _mm(ex[:, 128:256]),
                                         _mm(vext[:, kc, :]),
                                         start=(kc == 0), stop=(kc == d1))
                    for s, pes in ((0, pe0), (1, pe1)):
                        ts = d0 + s
                        rc = recp.tile([128, 1], dt)
                        nc.vector.reciprocal(rc, pes[:, 128:129])
                        en = encp.tile([128, 128], dt)
                        nc.vector.tensor_scalar_mul(en, pes[:, 0:128], rc)
                        ptt = pt2.tile([128, 128], dt)
                        nc.tensor.transpose(ptt, en, ident)
                        nc.scalar.copy(out=encT[:, n, ts, :], in_=ptt)

        # ---- phase 3: output projection, accumulate over heads ----
        with tc.tile_pool(name="wop", bufs=NL) as wop, \
             tc.tile_pool(name="outp", bufs=3) as outp, \
             tc.tile_pool(name="po3", bufs=1, space="PSUM") as po3:
            for dh in range(2):
                wo_sb = []
                for i in range(NL):
                    w = wop.tile([128, 1024], dt, tag="wo", name=f"wo_t{i}")
                    nc.sync.dma_start(out=w, in_=wo_d[i][:, dh * 1024:(dh + 1) * 1024])
                    wo_sb.append(w)
                for tg in range(2):
                    pos = [po3.tile([128, 1024], dt, tag=f"po{j}", name=f"po_t{j}")
                           for j in range(4)]
                    for n in range(NL):
                        for j in range(4):
                            ts = tg * 4 + j
                            for c2 in range(2):
                                nc.tensor.matmul(
                                    pos[j][:, c2 * 512:(c2 + 1) * 512],
                                    _mm(encT[:, n, ts, :]),
                                    _mm(wo_sb[n][:, c2 * 512:(c2 + 1) * 512]),
                                    start=(n == 0), stop=(n == NL - 1))
                    for j in range(4):
                        ts = tg * 4 + j
                        ob = outp.tile([128, 1024], dt)
                        nc.scalar.copy(out=ob, in_=pos[j])
                        nc.sync.dma_start(
                            out=out_d[ts * 128:(ts + 1) * 128,
                                      dh * 1024:(dh + 1) * 1024],
                            in_=ob)
    nc.compile()
    return nc


def make_in_maps(x, wq, wkv, wo, segment_pos, attn_mask):
    x = np.asarray(x, dtype=np.float32)
    wq = np.asarray(wq, dtype=np.float32)
    wkv = np.asarray(wkv, dtype=np.float32)
    wo = np.asarray(wo, dtype=np.float32)
    segment_pos = np.asarray(segment_pos)
    attn_mask = np.asarray(attn_mask)

    # rope-pair interleave permutation (see SHUF_MASK): lane j of quadrant qd
    # holds orig dim qd*16+(j%16) for lanes 0-15, 64+qd*16+(j%16) for 16-31.
    lanes = np.arange(HD)
    qd, lane = lanes // 32, lanes % 32
    f = qd * 16 + (lane % 16)
    perm = np.where(lane < 16, f, HHD + f)
    sgn = np.where(lane < 16, np.float32(-1.0), np.float32(1.0))

    wk = np.ascontiguousarray(wkv[0, 0][:, perm])
    wv = np.ascontiguousarray(wkv[1, 0])
    frac = (2.0 / HD) * np.arange(HHD, dtype=np.float32)
    timescale = (np.float32(10000.0) ** frac).astype(np.float32)
    scale = np.float32(HD ** -0.5)

    in_maps = []
    for c in range(8):
        b, g = c // 2, c % 2
        pos = segment_pos[b].astype(np.float32)
        sinus = pos[:, None] / timescale[None, :]          # [T, 64]
        cos = np.cos(sinus).astype(np.float32).T           # [64, T]
        sin = np.sin(sinus).astype(np.float32).T
        cosD = cos[f, :]                                   # [128, T]
        sinS = sgn[:, None] * sin[f, :]
        tri = np.ascontiguousarray(
            attn_mask[b, :128, :128].T.astype(np.float32))
        in_maps.append({
            "x": np.ascontiguousarray(x[b]),
            "wq": np.ascontiguousarray(wq[g * NL:(g + 1) * NL][:, :, perm]),
            "wk": wk,
            "wv": wv,
            "wo": np.ascontiguousarray(wo[g * NL:(g + 1) * NL]),
            "cosq": np.ascontiguousarray(cosD * scale),
            "sinq": np.ascontiguousarray(sinS * scale),
            "cosk": np.ascontiguousarray(cosD),
            "sink": np.ascontiguousarray(sinS),
            "tri": tri,
        })
    return in_maps


_NC_CACHE = None


def kernel(**inputs):
    global _NC_CACHE
    if _NC_CACHE is None:
        _NC_CACHE = build_nc()
    nc = _NC_CACHE
    in_maps = make_in_maps(
        inputs["x"], inputs["wq"], inputs["wkv"], inputs["wo"],
        inputs["segment_pos"], inputs["attn_mask"])
    res = run_bass_kernel_spmd(nc, in_maps, core_ids=list(range(8)))
    out = np.empty((B, T, D), dtype=np.float32)
    for b in range(B):
        out[b] = res.results[2 * b]["out"] + res.results[2 * b + 1]["out"]
    return out


# revision 7
# speedup vs baseline: 2.3641x; 2.3641x over previous
"""Trainium2 Bass kernel for MQA attention (B=4, T=1024, D=2048, 16 q-heads, 1 kv-head).

Sharding: 8 cores = 4 batches x 2 head-groups (8 query heads each).
Each core computes, for its batch b and head-group g:
  - x^T via PE transposes (contraction layouts need D on partitions)
  - k/v projections (shared single KV head, duplicated across the pair)
  - RoPE on q/k in [H, tok] layout using host-precomputed sin/cos tables
  - causal attention in transposed-logits layout (logits^T = [k, q]) so that
    PV needs no transposes; softmax denominator rides as a fused ones-column
    of the PV rhs; no max-subtraction (logits are bounded by construction)
  - output projection for its 8 heads -> partial [T, D]
Host sums the two partials per batch (the pair all-reduce) and stacks batches.

The SPMD program is identical on all cores; only the data differs.
"""

import numpy as np
import concourse.bass as bass
import concourse.mybir as mybir
from concourse import bacc
from concourse.tile import TileContext
from concourse.bass_utils import run_bass_kernel_spmd
from concourse.masks import make_identity
from contextlib import ExitStack

F32 = mybir.dt.float32
# Matmul compute dtype: float32 (exact) or float32r (fast fp32: 1 cyc/row at
# N>=256 vs 4 for fp32; 20-bit storage, 11-bit mantissa, ~1.2e-4 rel precision).
MM_DT = mybir.dt.float32r

B, T, D, NH, HD = 4, 1024, 2048, 16, 128
HHD = HD // 2          # 64, rope half
NL = NH // 2           # 8 heads per core
DC = D // 128          # 16 contraction chunks
TT = T // 128          # 8 token tiles
EXPAD = 256            # PV rhs width: [v (128) | ones (1) | zeros (127)]

# Rope-pair interleave: the H dim of q/k is permuted (consistently in wq/wk
# columns, host-side) so each rope pair (f, f+64) sits 16 lanes apart within
# one 32-partition quadrant; the rotate-half becomes a stream_shuffle.
SHUF_MASK = list(range(16, 32)) + list(range(16))


def _rope(nc, out, pin, cos, sin, tmp, stage):
    """RoPE in permuted [H, tok] layout. pin: [128, W] (psum), cos: duplicated
    cos table, sin: sign-baked sin table (-sin on first-half lanes, +sin on
    second-half lanes), tmp/stage: [128, W] f32 sbuf scratch.
    out = pin * cos + shuffle16(pin) * sin; only the final add writes the
    (possibly fp32r) out tile so DVE inputs stay uniformly f32.
    """
    nc.vector.stream_shuffle(tmp, pin, SHUF_MASK)
    nc.vector.tensor_mul(stage, pin, cos)
    nc.vector.tensor_mul(tmp, tmp, sin)
    nc.vector.tensor_add(out, stage, tmp)


def _round_fp32r(a):
    """Round f32 array to fp32r (11-bit mantissa, RNE), keeping f32 storage."""
    if MM_DT == F32:
        return np.ascontiguousarray(a, dtype=np.float32)
    u = np.ascontiguousarray(a, dtype=np.float32).view(np.uint32)
    lsb = (u >> np.uint32(12)) & np.uint32(1)
    r = ((u + np.uint32(0x7FF) + lsb) & np.uint32(0xFFFFF000)).view(np.float32)
    return np.ascontiguousarray(r)


def build_nc():
    nc = bacc.Bacc("TRN2", target_bir_lowering=False, debug=False, num_devices=8)
    dt = F32
    x_d = nc.dram_tensor("x", [T, D], dt, kind="ExternalInput").ap()
    wq_d = nc.dram_tensor("wq", [NL, D, HD], MM_DT, kind="ExternalInput").ap()
    wk_d = nc.dram_tensor("wk", [D, HD], MM_DT, kind="ExternalInput").ap()
    wv_d = nc.dram_tensor("wv", [D, HD], MM_DT, kind="ExternalInput").ap()
    wo_d = nc.dram_tensor("wo", [NL, HD, D], MM_DT, kind="ExternalInput").ap()
    cosq_d = nc.dram_tensor("cosq", [128, T], dt, kind="ExternalInput").ap()
    sinq_d = nc.dram_tensor("sinq", [128, T], dt, kind="ExternalInput").ap()
    cosk_d = nc.dram_tensor("cosk", [128, T], dt, kind="ExternalInput").ap()
    sink_d = nc.dram_tensor("sink", [128, T], dt, kind="ExternalInput").ap()
    tri_d = nc.dram_tensor("tri", [128, 128], MM_DT, kind="ExternalInput").ap()
    out_d = nc.dram_tensor("out", [T, D], dt, kind="ExternalOutput").ap()

    with TileContext(nc) as tc, ExitStack() as ctx:
        singles = ctx.enter_context(tc.tile_pool(name="singles", bufs=1))
        ident = singles.tile([128, 128], dt)
        make_identity(nc, ident)
        tri = singles.tile([128, 128], MM_DT)
        nc.sync.dma_start(out=tri, in_=tri_d)
        cosq = singles.tile([128, T], dt)
        sinq = singles.tile([128, T], dt)
        cosk = singles.tile([128, T], dt)
        sink = singles.tile([128, T], dt)
        nc.sync.dma_start(out=cosq, in_=cosq_d)
        nc.sync.dma_start(out=sinq, in_=sinq_d)
        nc.sync.dma_start(out=cosk, in_=cosk_d)
        nc.sync.dma_start(out=sink, in_=sink_d)
        wk_sb = singles.tile([128, DC, HD], MM_DT)
        wv_sb = singles.tile([128, DC, HD], MM_DT)
        nc.sync.dma_start(out=wk_sb, in_=wk_d.rearrange("(c p) h -> p c h", p=128))
        nc.sync.dma_start(out=wv_sb, in_=wv_d.rearrange("(c p) h -> p c h", p=128))
        xT = singles.tile([128, DC, T], MM_DT)        # x transposed, 8MB
        kT = singles.tile([128, T], MM_DT)            # roped k^T
        vext = singles.tile([128, TT, EXPAD], MM_DT)  # v | ones | zeros
        onecol = singles.tile([128, 1], dt)
        nc.vector.memset(onecol, 1.0)
        zcol = singles.tile([128, EXPAD - HD - 1], dt)
        nc.vector.memset(zcol, 0.0)
        encT = singles.tile([128, NL, TT, 128], MM_DT)  # encoded^T per head, 4MB

        # ---- phase 0: load x, build xT ----
        with tc.tile_pool(name="xload", bufs=2) as xload, \
             tc.tile_pool(name="ptr", bufs=2, space="PSUM") as ptr:
            for t in range(TT):
                xn = xload.tile([128, D], dt)
                nc.sync.dma_start(out=xn, in_=x_d[t * 128:(t + 1) * 128, :])
                for c in range(DC):
                    pt = ptr.tile([128, 128], dt)
                    nc.tensor.transpose(pt, xn[:, c * 128:(c + 1) * 128], ident)
                    nc.scalar.copy(out=xT[:, c, t * 128:(t + 1) * 128], in_=pt)

        # ---- phase 1: k^T (roped) and v_ext ----
        with tc.tile_pool(name="pk1", bufs=2, space="PSUM") as pk1, \
             tc.tile_pool(name="pv1", bufs=2, space="PSUM") as pv1, \
             tc.tile_pool(name="ktmp", bufs=2) as ktmp:
            for th in range(2):
                sl = slice(th * 512, (th + 1) * 512)
                pk = pk1.tile([128, 512], dt)
                for c in range(DC):
                    nc.tensor.matmul(pk, (wk_sb[:, c, :]), (xT[:, c, sl]),
                                     start=(c == 0), stop=(c == DC - 1))
                tmp = ktmp.tile([128, 512], dt)
                stage = ktmp.tile([128, 512], dt, tag="stage", name="kstage")
                _rope(nc, kT[:, sl], pk[:, :], cosk[:, sl], sink[:, sl], tmp,
                      stage)
            for tb in range(TT):
                pv = pv1.tile([128, 128], dt)
                for c in range(DC):
                    nc.tensor.matmul(pv, (xT[:, c, tb * 128:(tb + 1) * 128]),
                                     (wv_sb[:, c, :]),
                                     start=(c == 0), stop=(c == DC - 1))
                nc.scalar.copy(out=vext[:, tb, 0:128], in_=pv)
                nc.scalar.copy(out=vext[:, tb, 128:129], in_=onecol)
                nc.scalar.copy(out=vext[:, tb, 129:EXPAD], in_=zcol)

        # ---- phase 2: per-head q-proj + rope + causal attention ----
        with tc.tile_pool(name="wqp", bufs=2) as wqp, \
             tc.tile_pool(name="qtp", bufs=2) as qtp, \
             tc.tile_pool(name="ropet", bufs=2) as ropet, \
             tc.tile_pool(name="expp", bufs=3) as expp, \
             tc.tile_pool(name="encp", bufs=2) as encp, \
             tc.tile_pool(name="recp", bufs=2) as recp, \
             tc.tile_pool(name="pq2", bufs=2, space="PSUM") as pq2, \
             tc.tile_pool(name="pl2", bufs=2, space="PSUM") as pl2, \
             tc.tile_pool(name="pe2", bufs=1, space="PSUM") as pe2, \
             tc.tile_pool(name="pt2", bufs=2, space="PSUM") as pt2:
            for n in range(NL):
                wq_sb = wqp.tile([128, DC, HD], MM_DT)
                nc.sync.dma_start(out=wq_sb,
                                  in_=wq_d[n].rearrange("(c p) h -> p c h", p=128))
                qT = qtp.tile([128, T], MM_DT)
                for th in range(2):
                    sl = slice(th * 512, (th + 1) * 512)
                    pq = pq2.tile([128, 512], dt)
                    for c in range(DC):
                        nc.tensor.matmul(pq, (wq_sb[:, c, :]), (xT[:, c, sl]),
                                         start=(c == 0), stop=(c == DC - 1))
                    tmp = ropet.tile([128, 512], dt)
                    stage = ropet.tile([128, 512], dt, tag="qstage",
                                       name="qstage")
                    _rope(nc, qT[:, sl], pq, cosq[:, sl], sinq[:, sl], tmp,
                          stage)
                for qb in range(4):          # q blocks of 256 rows
                    R = qb * 256
                    d0 = R // 128            # diagonal chunk of sub0
                    d1 = d0 + 1              # diagonal chunk of sub1 (last)
                    pe0 = pe2.tile([128, EXPAD], dt, tag="pe0", name="pe0")
                    pe1 = pe2.tile([128, EXPAD], dt, tag="pe1", name="pe1")
                    for kc in range(d1 + 1):
                        plt = pl2.tile([128, 256], dt)
                        nc.tensor.matmul(plt, (kT[:, kc * 128:(kc + 1) * 128]),
                                         (qT[:, R:R + 256]),
                                         start=True, stop=True)
                        ex = expp.tile([128, 256], MM_DT)
                        nc.scalar.activation(out=ex, in_=plt,
                                             func=mybir.ActivationFunctionType.Exp)
                        if kc == d0:
                            nc.vector.tensor_mul(ex[:, 0:128], ex[:, 0:128], tri)
                        if kc == d1:
                            nc.vector.tensor_mul(ex[:, 128:256], ex[:, 128:256], tri)
                        if kc <= d0:
                            nc.tensor.matmul(pe0, (ex[:, 0:128]),
                                             (vext[:, kc, :]),
                                             start=(kc == 0), stop=(kc == d0))
                        nc.tensor.matmul(pe1, (ex[:, 128:256]),
                                         (vext[:, kc, :]),
                                         start=(kc == 0), stop=(kc == d1))
                    for s, pes in ((0, pe0), (1, pe1)):
                        ts = d0 + s
                        rc = recp.tile([128, 1], dt)
                        nc.vector.reciprocal(rc, pes[:, 128:129])
                        en = encp.tile([128, 128], dt)
                        nc.vector.tensor_scalar_mul(en, pes[:, 0:128], rc)
                        ptt = pt2.tile([128, 128], dt)
                        nc.tensor.transpose(ptt, en, ident)
                        nc.scalar.copy(out=encT[:, n, ts, :], in_=ptt)

        # ---- phase 3: output projection, accumulate over heads ----
        with tc.tile_pool(name="wop", bufs=NL) as wop, \
             tc.tile_pool(name="outp", bufs=3) as outp, \
             tc.tile_pool(name="po3", bufs=1, space="PSUM") as po3:
            for dh in range(2):
                wo_sb = []
                for i in range(NL):
                    w = wop.tile([128, 1024], MM_DT, tag="wo", name=f"wo_t{i}")
                    nc.sync.dma_start(out=w, in_=wo_d[i][:, dh * 1024:(dh + 1) * 1024])
                    wo_sb.append(w)
                for tg in range(2):
                    pos = [po3.tile([128, 1024], dt, tag=f"po{j}", name=f"po_t{j}")
                           for j in range(4)]
                    for n in range(NL):
                        for j in range(4):
                            ts = tg * 4 + j
                            for c2 in range(2):
                                nc.tensor.matmul(
                                    pos[j][:, c2 * 512:(c2 + 1) * 512],
                                    (encT[:, n, ts, :]),
                                    (wo_sb[n][:, c2 * 512:(c2 + 1) * 512]),
                                    start=(n == 0), stop=(n == NL - 1))
                    for j in range(4):
                        ts = tg * 4 + j
                        ob = outp.tile([128, 1024], dt)
                        nc.scalar.copy(out=ob, in_=pos[j])
                        nc.sync.dma_start(
                            out=out_d[ts * 128:(ts + 1) * 128,
                                      dh * 1024:(dh + 1) * 1024],
                            in_=ob)
    nc.compile()
    return nc


def make_in_maps(x, wq, wkv, wo, segment_pos, attn_mask):
    x = np.asarray(x, dtype=np.float32)
    wq = np.asarray(wq, dtype=np.float32)
    wkv = np.asarray(wkv, dtype=np.float32)
    wo = np.asarray(wo, dtype=np.float32)
    segment_pos = np.asarray(segment_pos)
    attn_mask = np.asarray(attn_mask)

    # rope-pair interleave permutation (see SHUF_MASK): lane j of quadrant qd
    # holds orig dim qd*16+(j%16) for lanes 0-15, 64+qd*16+(j%16) for 16-31.
    lanes = np.arange(HD)
    qd, lane = lanes // 32, lanes % 32
    f = qd * 16 + (lane % 16)
    perm = np.where(lane < 16, f, HHD + f)
    sgn = np.where(lane < 16, np.float32(-1.0), np.float32(1.0))

    wk = _round_fp32r(wkv[0, 0][:, perm])
    wv = _round_fp32r(wkv[1, 0])
    frac = (2.0 / HD) * np.arange(HHD, dtype=np.float32)
    timescale = (np.float32(10000.0) ** frac).astype(np.float32)
    scale = np.float32(HD ** -0.5)

    in_maps = []
    for c in range(8):
        b, g = c // 2, c % 2
        pos = segment_pos[b].astype(np.float32)
        sinus = pos[:, None] / timescale[None, :]          # [T, 64]
        cos = np.cos(sinus).astype(np.float32).T           # [64, T]
        sin = np.sin(sinus).astype(np.float32).T
        cosD = cos[f, :]                                   # [128, T]
        sinS = sgn[:, None] * sin[f, :]
        tri = np.ascontiguousarray(
            attn_mask[b, :128, :128].T.astype(np.float32))  # 0/1: fp32r-exact
        in_maps.append({
            "x": np.ascontiguousarray(x[b]),
            "wq": _round_fp32r(wq[g * NL:(g + 1) * NL][:, :, perm]),
            "wk": wk,
            "wv": wv,
            "wo": _round_fp32r(wo[g * NL:(g + 1) * NL]),
            "cosq": np.ascontiguousarray(cosD * scale),
            "sinq": np.ascontiguousarray(sinS * scale),
            "cosk": np.ascontiguousarray(cosD),
            "sink": np.ascontiguousarray(sinS),
            "tri": tri,
        })
    return in_maps


_NC_CACHE = None


def kernel(**inputs):
    global _NC_CACHE
    if _NC_CACHE is None:
        _NC_CACHE = build_nc()
    nc = _NC_CACHE
    in_maps = make_in_maps(
        inputs["x"], inputs["wq"], inputs["wkv"], inputs["wo"],
        inputs["segment_pos"], inputs["attn_mask"])
    res = run_bass_kernel_spmd(nc, in_maps, core_ids=list(range(8)))
    out = np.empty((B, T, D), dtype=np.float32)
    for b in range(B):
        out[b] = res.results[2 * b]["out"] + res.results[2 * b + 1]["out"]
    return out


# revision 10
# speedup vs baseline: 2.7409x; 1.1594x over previous
"""Trainium2 Bass kernel for MQA attention (B=4, T=1024, D=2048, 16 q-heads, 1 kv-head).

Sharding: 8 cores = 4 batches x 2 head-groups (8 query heads each).
Each core computes, for its batch b and head-group g:
  - x^T via PE transposes (contraction layouts need D on partitions)
  - k/v projections (shared single KV head, duplicated across the pair)
  - RoPE on q/k in [H, tok] layout using host-precomputed sin/cos tables
  - causal attention in transposed-logits layout (logits^T = [k, q]) so that
    PV needs no transposes; softmax denominator rides as a fused ones-column
    of the PV rhs; no max-subtraction (logits are bounded by construction)
  - output projection for its 8 heads -> partial [T, D]
Host sums the two partials per batch (the pair all-reduce) and stacks batches.

Matmul inputs are bf16 (f32 PSUM accumulation; TensorE gets fast-weight-load
at bf16, which fp32/fp32r cannot use); softmax statistics, normalization and
the final output stay f32.

The SPMD program is identical on all cores; only the data differs.
"""

import numpy as np
import ml_dtypes
import concourse.bass as bass
import concourse.mybir as mybir
from concourse import bacc
from concourse.tile import TileContext
from concourse.bass_utils import run_bass_kernel_spmd
from concourse.masks import make_identity
from contextlib import ExitStack

F32 = mybir.dt.float32
BF16 = mybir.dt.bfloat16
NP_BF16 = ml_dtypes.bfloat16

B, T, D, NH, HD = 4, 1024, 2048, 16, 128
HHD = HD // 2          # 64, rope half
NL = NH // 2           # 8 heads per core
DC = D // 128          # 16 contraction chunks
TT = T // 128          # 8 token tiles
EXPAD = 256            # PV rhs width: [v (128) | ones (1) | zeros (127)]

# Rope-pair interleave: the H dim of q/k is permuted (consistently in wq/wk
# columns, host-side) so each rope pair (f, f+64) sits 16 lanes apart within
# one 32-partition quadrant; the rotate-half becomes a stream_shuffle.
SHUF_MASK = list(range(16, 32)) + list(range(16))


def _rope(nc, out, pin, cos, sin, tmp, stage):
    """RoPE in permuted [H, tok] layout. pin: [128, W] (psum f32), cos:
    duplicated cos table, sin: sign-baked sin table (-sin on first-half lanes,
    +sin on second-half lanes), tmp/stage: [128, W] f32 sbuf scratch.
    out (bf16) = pin * cos + shuffle16(pin) * sin.
    """
    nc.vector.stream_shuffle(tmp, pin, SHUF_MASK)
    nc.vector.tensor_mul(stage, pin, cos)
    nc.vector.tensor_mul(tmp, tmp, sin)
    nc.vector.tensor_add(out, stage, tmp)


def build_nc():
    nc = bacc.Bacc("TRN2", target_bir_lowering=False, debug=False, num_devices=8)
    dt = F32
    x_d = nc.dram_tensor("x", [T, D], BF16, kind="ExternalInput").ap()
    wq_d = nc.dram_tensor("wq", [NL, D, HD], BF16, kind="ExternalInput").ap()
    wk_d = nc.dram_tensor("wk", [D, HD], BF16, kind="ExternalInput").ap()
    wv_d = nc.dram_tensor("wv", [D, HD], BF16, kind="ExternalInput").ap()
    wo_d = nc.dram_tensor("wo", [NL, HD, D], BF16, kind="ExternalInput").ap()
    cosq_d = nc.dram_tensor("cosq", [128, T], dt, kind="ExternalInput").ap()
    sinq_d = nc.dram_tensor("sinq", [128, T], dt, kind="ExternalInput").ap()
    cosk_d = nc.dram_tensor("cosk", [128, T], dt, kind="ExternalInput").ap()
    sink_d = nc.dram_tensor("sink", [128, T], dt, kind="ExternalInput").ap()
    tri_d = nc.dram_tensor("tri", [128, 128], BF16, kind="ExternalInput").ap()
    out_d = nc.dram_tensor("out", [T, D], dt, kind="ExternalOutput").ap()

    with TileContext(nc) as tc, ExitStack() as ctx:
        singles = ctx.enter_context(tc.tile_pool(name="singles", bufs=1))
        xnp = ctx.enter_context(tc.tile_pool(name="xnp", bufs=TT))

        # x tiles stream in first, on the sync DMA queue; everything else
        # loads on the gpsimd queue so it cannot delay the x-dependent PE work.
        xns = []
        for t in range(TT):
            xn = xnp.tile([128, D], BF16, tag="xn", name=f"xn{t}")
            nc.sync.dma_start(out=xn, in_=x_d[t * 128:(t + 1) * 128, :])
            xns.append(xn)

        ident = singles.tile([128, 128], BF16)
        make_identity(nc, ident)
        tri = singles.tile([128, 128], BF16)
        nc.gpsimd.dma_start(out=tri, in_=tri_d)
        cosq = singles.tile([128, T], dt)
        sinq = singles.tile([128, T], dt)
        cosk = singles.tile([128, T], dt)
        sink = singles.tile([128, T], dt)
        nc.gpsimd.dma_start(out=cosq, in_=cosq_d)
        nc.gpsimd.dma_start(out=sinq, in_=sinq_d)
        nc.gpsimd.dma_start(out=cosk, in_=cosk_d)
        nc.gpsimd.dma_start(out=sink, in_=sink_d)
        wk_sb = singles.tile([128, DC, HD], BF16)
        wv_sb = singles.tile([128, DC, HD], BF16)
        nc.gpsimd.dma_start(out=wk_sb, in_=wk_d.rearrange("(c p) h -> p c h", p=128))
        nc.gpsimd.dma_start(out=wv_sb, in_=wv_d.rearrange("(c p) h -> p c h", p=128))
        onecol = singles.tile([128, 1], dt)
        nc.vector.memset(onecol, 1.0)
        zcol = singles.tile([128, EXPAD - HD - 1], dt)
        nc.vector.memset(zcol, 0.0)

        xT = singles.tile([128, DC, T], BF16)      # x transposed, 4MB
        kT = singles.tile([128, T], BF16)          # roped k^T
        vext = singles.tile([128, TT, EXPAD], BF16)  # v | ones | zeros
        encT = singles.tile([128, NL, TT, 128], BF16)  # encoded^T per head, 2MB

        # ---- phase 0: build xT ----
        with tc.tile_pool(name="ptr", bufs=2, space="PSUM") as ptr:
            for t in range(TT):
                for c in range(DC):
                    pt = ptr.tile([128, 128], BF16)
                    nc.tensor.transpose(pt, xns[t][:, c * 128:(c + 1) * 128],
                                        ident)
                    nc.scalar.copy(out=xT[:, c, t * 128:(t + 1) * 128], in_=pt)

        # ---- phase 1: k^T (roped) and v_ext ----
        with tc.tile_pool(name="pk1", bufs=2, space="PSUM") as pk1, \
             tc.tile_pool(name="pv1", bufs=2, space="PSUM") as pv1, \
             tc.tile_pool(name="ktmp", bufs=2) as ktmp:
            for th in range(2):
                sl = slice(th * 512, (th + 1) * 512)
                pk = pk1.tile([128, 512], dt)
                for c in range(DC):
                    nc.tensor.matmul(pk, wk_sb[:, c, :], xT[:, c, sl],
                                     start=(c == 0), stop=(c == DC - 1))
                tmp = ktmp.tile([128, 512], dt)
                stage = ktmp.tile([128, 512], dt, tag="stage", name="kstage")
                _rope(nc, kT[:, sl], pk[:, :], cosk[:, sl], sink[:, sl], tmp,
                      stage)
            for tb in range(TT):
                pv = pv1.tile([128, 128], dt)
                for c in range(DC):
                    nc.tensor.matmul(pv, xT[:, c, tb * 128:(tb + 1) * 128],
                                     wv_sb[:, c, :],
                                     start=(c == 0), stop=(c == DC - 1))
                nc.scalar.copy(out=vext[:, tb, 0:128], in_=pv)
                nc.scalar.copy(out=vext[:, tb, 128:129], in_=onecol)
                nc.scalar.copy(out=vext[:, tb, 129:EXPAD], in_=zcol)

        # ---- phase 2: per-head q-proj + rope + causal attention ----
        with tc.tile_pool(name="wqp", bufs=2) as wqp, \
             tc.tile_pool(name="qtp", bufs=2) as qtp, \
             tc.tile_pool(name="ropet", bufs=2) as ropet, \
             tc.tile_pool(name="expp", bufs=3) as expp, \
             tc.tile_pool(name="encp", bufs=2) as encp, \
             tc.tile_pool(name="recp", bufs=2) as recp, \
             tc.tile_pool(name="pq2", bufs=2, space="PSUM") as pq2, \
             tc.tile_pool(name="pl2", bufs=2, space="PSUM") as pl2, \
             tc.tile_pool(name="pe2", bufs=1, space="PSUM") as pe2, \
             tc.tile_pool(name="pt2", bufs=2, space="PSUM") as pt2:
            for n in range(NL):
                wq_sb = wqp.tile([128, DC, HD], BF16)
                nc.gpsimd.dma_start(out=wq_sb,
                                    in_=wq_d[n].rearrange("(c p) h -> p c h",
                                                          p=128))
                qT = qtp.tile([128, T], BF16)
                for th in range(2):
                    sl = slice(th * 512, (th + 1) * 512)
                    pq = pq2.tile([128, 512], dt)
                    for c in range(DC):
                        nc.tensor.matmul(pq, wq_sb[:, c, :], xT[:, c, sl],
                                         start=(c == 0), stop=(c == DC - 1))
                    tmp = ropet.tile([128, 512], dt)
                    stage = ropet.tile([128, 512], dt, tag="qstage",
                                       name="qstage")
                    _rope(nc, qT[:, sl], pq, cosq[:, sl], sinq[:, sl], tmp,
                          stage)
                for qb in range(4):          # q blocks of 256 rows
                    R = qb * 256
                    d0 = R // 128            # diagonal chunk of sub0
                    d1 = d0 + 1              # diagonal chunk of sub1 (last)
                    pe0 = pe2.tile([128, EXPAD], dt, tag="pe0", name="pe0")
                    pe1 = pe2.tile([128, EXPAD], dt, tag="pe1", name="pe1")
                    for kc in range(d1 + 1):
                        plt = pl2.tile([128, 256], dt)
                        nc.tensor.matmul(plt, kT[:, kc * 128:(kc + 1) * 128],
                                         qT[:, R:R + 256],
                                         start=True, stop=True)
                        ex = expp.tile([128, 256], BF16)
                        nc.scalar.activation(out=ex, in_=plt,
                                             func=mybir.ActivationFunctionType.Exp)
                        if kc == d0:
                            nc.vector.tensor_mul(ex[:, 0:128], ex[:, 0:128], tri)
                        if kc == d1:
                            nc.vector.tensor_mul(ex[:, 128:256], ex[:, 128:256],
                                                 tri)
                        if kc <= d0:
                            nc.tensor.matmul(pe0, ex[:, 0:128],
                                             vext[:, kc, :],
                                             start=(kc == 0), stop=(kc == d0))
                        nc.tensor.matmul(pe1, ex[:, 128:256],
                                         vext[:, kc, :],
                                         start=(kc == 0), stop=(kc == d1))
                    for s, pes in ((0, pe0), (1, pe1)):
                        ts = d0 + s
                        rc = recp.tile([128, 1], dt)
                        nc.vector.reciprocal(rc, pes[:, 128:129])
                        en = encp.tile([128, 128], BF16)
                        nc.vector.tensor_scalar_mul(en, pes[:, 0:128], rc)
                        ptt = pt2.tile([128, 128], BF16)
                        nc.tensor.transpose(ptt, en, ident)
                        nc.vector.tensor_copy(out=encT[:, n, ts, :], in_=ptt)

        # ---- phase 3: output projection, accumulate over heads ----
        with tc.tile_pool(name="wop", bufs=2 * NL) as wop, \
             tc.tile_pool(name="outp", bufs=4) as outp, \
             tc.tile_pool(name="po3", bufs=1, space="PSUM") as po3:
            for dh in range(2):
                wo_sb = []
                for i in range(NL):
                    w = wop.tile([128, 1024], BF16, tag="wo",
                                 name=f"wo_t{dh}_{i}")
                    nc.gpsimd.dma_start(out=w,
                                        in_=wo_d[i][:, dh * 1024:(dh + 1) * 1024])
                    wo_sb.append(w)
                for tg in range(2):
                    pos = [po3.tile([128, 1024], dt, tag=f"po{j}",
                                    name=f"po_t{j}") for j in range(4)]
                    for n in range(NL):
                        for j in range(4):
                            ts = tg * 4 + j
                            for c2 in range(2):
                                nc.tensor.matmul(
                                    pos[j][:, c2 * 512:(c2 + 1) * 512],
                                    encT[:, n, ts, :],
                                    wo_sb[n][:, c2 * 512:(c2 + 1) * 512],
                                    start=(n == 0), stop=(n == NL - 1))
                    for j in range(4):
                        ts = tg * 4 + j
                        ob = outp.tile([128, 1024], dt)
                        nc.vector.tensor_copy(out=ob, in_=pos[j])
                        nc.sync.dma_start(
                            out=out_d[ts * 128:(ts + 1) * 128,
                                      dh * 1024:(dh + 1) * 1024],
                            in_=ob)
    nc.compile()
    return nc


def make_in_maps(x, wq, wkv, wo, segment_pos, attn_mask):
    x = np.asarray(x, dtype=np.float32)
    wq = np.asarray(wq, dtype=np.float32)
    wkv = np.asarray(wkv, dtype=np.float32)
    wo = np.asarray(wo, dtype=np.float32)
    segment_pos = np.asarray(segment_pos)
    attn_mask = np.asarray(attn_mask)

    # rope-pair interleave permutation (see SHUF_MASK): lane j of quadrant qd
    # holds orig dim qd*16+(j%16) for lanes 0-15, 64+qd*16+(j%16) for 16-31.
    lanes = np.arange(HD)
    qd, lane = lanes // 32, lanes % 32
    f = qd * 16 + (lane % 16)
    perm = np.where(lane < 16, f, HHD + f)
    sgn = np.where(lane < 16, np.float32(-1.0), np.float32(1.0))

    wk = np.ascontiguousarray(wkv[0, 0][:, perm].astype(NP_BF16))
    wv = np.ascontiguousarray(wkv[1, 0].astype(NP_BF16))
    frac = (2.0 / HD) * np.arange(HHD, dtype=np.float32)
    timescale = (np.float32(10000.0) ** frac).astype(np.float32)
    scale = np.float32(HD ** -0.5)

    in_maps = []
    for c in range(8):
        b, g = c // 2, c % 2
        pos = segment_pos[b].astype(np.float32)
        sinus = pos[:, None] / timescale[None, :]          # [T, 64]
        cos = np.cos(sinus).astype(np.float32).T           # [64, T]
        sin = np.sin(sinus).astype(np.float32).T
        cosD = cos[f, :]                                   # [128, T]
        sinS = sgn[:, None] * sin[f, :]
        tri = np.ascontiguousarray(
            attn_mask[b, :128, :128].T.astype(NP_BF16))    # 0/1: bf16-exact
        in_maps.append({
            "x": np.ascontiguousarray(x[b].astype(NP_BF16)),
            "wq": np.ascontiguousarray(
                wq[g * NL:(g + 1) * NL][:, :, perm].astype(NP_BF16)),
            "wk": wk,
            "wv": wv,
            "wo": np.ascontiguousarray(
                wo[g * NL:(g + 1) * NL].astype(NP_BF16)),
            "cosq": np.ascontiguousarray(cosD * scale),
            "sinq": np.ascontiguousarray(sinS * scale),
            "cosk": np.ascontiguousarray(cosD),
            "sink": np.ascontiguousarray(sinS),
            "tri": tri,
        })
    return in_maps


_NC_CACHE = None


def kernel(**inputs):
    global _NC_CACHE
    if _NC_CACHE is None:
        _NC_CACHE = build_nc()
    nc = _NC_CACHE
    in_maps = make_in_maps(
        inputs["x"], inputs["wq"], inputs["wkv"], inputs["wo"],
        inputs["segment_pos"], inputs["attn_mask"])
    res = run_bass_kernel_spmd(nc, in_maps, core_ids=list(range(8)))
    out = np.empty((B, T, D), dtype=np.float32)
    for b in range(B):
        out[b] = res.results[2 * b]["out"] + res.results[2 * b + 1]["out"]
    return out


# revision 13
# speedup vs baseline: 3.3946x; 1.2385x over previous
"""Trainium2 Bass kernel for MQA attention (B=4, T=1024, D=2048, 16 q-heads, 1 kv-head).

Sharding: 8 cores = 4 batches x 2 head-groups (8 query heads each).
Each core computes, for its batch b and head-group g:
  - x^T via DMA xbar transposes (contraction layouts need D on partitions)
  - k/v projections (shared single KV head, duplicated across the pair)
  - RoPE on q/k in [H, tok] layout using host-precomputed sin/cos tables
  - causal attention in transposed-logits layout (logits^T = [k, q]) so that
    PV needs no transposes; softmax denominator rides as a fused ones-column
    of the PV rhs; no max-subtraction (logits are bounded by construction)
  - output projection for its 8 heads -> partial [T, D]
Host sums the two partials per batch (the pair all-reduce) and stacks batches.

Matmul inputs are bf16 (f32 PSUM accumulation; TensorE gets fast-weight-load
at bf16, which fp32/fp32r cannot use); softmax statistics, normalization and
the final output stay f32.

The SPMD program is identical on all cores; only the data differs.
"""

import numpy as np
import ml_dtypes
import concourse.bass as bass
import concourse.mybir as mybir
from concourse import bacc
from concourse.tile import TileContext
from concourse.bass_utils import run_bass_kernel_spmd
from concourse.masks import make_identity
from contextlib import ExitStack

F32 = mybir.dt.float32
BF16 = mybir.dt.bfloat16
NP_BF16 = ml_dtypes.bfloat16

B, T, D, NH, HD = 4, 1024, 2048, 16, 128
HHD = HD // 2          # 64, rope half
NL = NH // 2           # 8 heads per core
DC = D // 128          # 16 contraction chunks
TT = T // 128          # 8 token tiles
EXPAD = 129            # PV rhs width: [v (128) | ones (1)]

# Rope-pair interleave: the H dim of q/k is permuted (consistently in wq/wk
# columns, host-side) so each rope pair (f, f+64) sits 16 lanes apart within
# one 32-partition quadrant; the rotate-half becomes a stream_shuffle.
SHUF_MASK = list(range(16, 32)) + list(range(16))


def _rope(nc, out, pin, cos, sin, tmp, stage):
    """RoPE in permuted [H, tok] layout. pin: [128, W] (psum f32), cos:
    duplicated cos table, sin: sign-baked sin table (-sin on first-half lanes,
    +sin on second-half lanes), tmp/stage: [128, W] f32 sbuf scratch.
    out (bf16) = pin * cos + shuffle16(pin) * sin.
    """
    nc.vector.stream_shuffle(tmp, pin, SHUF_MASK)
    nc.vector.tensor_mul(stage, pin, cos)
    nc.vector.tensor_mul(tmp, tmp, sin)
    nc.vector.tensor_add(out, stage, tmp)


def build_nc():
    nc = bacc.Bacc("TRN2", target_bir_lowering=False, debug=False, num_devices=8)
    dt = F32
    x_d = nc.dram_tensor("x", [T, D], BF16, kind="ExternalInput").ap()
    wq_d = nc.dram_tensor("wq", [NL, D, HD], BF16, kind="ExternalInput").ap()
    wk_d = nc.dram_tensor("wk", [D, HD], BF16, kind="ExternalInput").ap()
    wv_d = nc.dram_tensor("wv", [D, HD], BF16, kind="ExternalInput").ap()
    wo_d = nc.dram_tensor("wo", [NL, HD, D], BF16, kind="ExternalInput").ap()
    cosq_d = nc.dram_tensor("cosq", [128, T], dt, kind="ExternalInput").ap()
    sinq_d = nc.dram_tensor("sinq", [128, T], dt, kind="ExternalInput").ap()
    cosk_d = nc.dram_tensor("cosk", [128, T], dt, kind="ExternalInput").ap()
    sink_d = nc.dram_tensor("sink", [128, T], dt, kind="ExternalInput").ap()
    tri_d = nc.dram_tensor("tri", [128, 128], BF16, kind="ExternalInput").ap()
    out_d = nc.dram_tensor("out", [T, D], dt, kind="ExternalOutput").ap()

    with TileContext(nc) as tc, ExitStack() as ctx:
        singles = ctx.enter_context(tc.tile_pool(name="singles", bufs=1))

        xT = singles.tile([128, DC, T], BF16)      # x transposed, 4MB
        kT = singles.tile([128, T], BF16)          # roped k^T
        vext = singles.tile([128, TT, EXPAD], BF16)  # v | ones column
        encT = singles.tile([128, NL, TT, 128], BF16)  # encoded^T per head, 2MB

        # xT streams in first via xbar-transpose DMAs on the sync queue (all
        # transpose-mode DMAs stay on sync; plain DMAs use gpsimd/vector so
        # the xbar mode never thrashes within a queue).
        for c in range(DC):
            nc.sync.dma_start_transpose(out=xT[:, c, :],
                                        in_=x_d[:, c * 128:(c + 1) * 128])

        ident = singles.tile([128, 128], BF16)
        make_identity(nc, ident)
        wk_sb = singles.tile([128, DC, HD], BF16)
        wv_sb = singles.tile([128, DC, HD], BF16)
        nc.sync.dma_start(out=wk_sb, in_=wk_d.rearrange("(c p) h -> p c h", p=128))
        nc.sync.dma_start(out=wv_sb, in_=wv_d.rearrange("(c p) h -> p c h", p=128))
        cosq = singles.tile([128, T], dt)
        sinq = singles.tile([128, T], dt)
        cosk = singles.tile([128, T], dt)
        sink = singles.tile([128, T], dt)
        nc.sync.dma_start(out=cosk, in_=cosk_d)
        nc.sync.dma_start(out=sink, in_=sink_d)
        nc.sync.dma_start(out=cosq, in_=cosq_d)
        nc.sync.dma_start(out=sinq, in_=sinq_d)
        tri = singles.tile([128, 128], BF16)
        nc.sync.dma_start(out=tri, in_=tri_d)
        onecol = singles.tile([128, 1], dt)
        nc.vector.memset(onecol, 1.0)

        # all of wq / wo prefetches upfront (6MB of SBUF is cheap at bf16)
        wqp = ctx.enter_context(tc.tile_pool(name="wqp", bufs=NL))
        wop = ctx.enter_context(tc.tile_pool(name="wop", bufs=2 * NL))
        wq_sbs = []
        for n in range(NL):
            w = wqp.tile([128, DC, HD], BF16, tag="wq", name=f"wq_t{n}")
            nc.sync.dma_start(out=w,
                              in_=wq_d[n].rearrange("(c p) h -> p c h", p=128))
            wq_sbs.append(w)
        wo_sbs = []
        for dh in range(2):
            for i in range(NL):
                w = wop.tile([128, 1024], BF16, tag="wo", name=f"wo_t{dh}_{i}")
                nc.sync.dma_start(out=w,
                                  in_=wo_d[i][:, dh * 1024:(dh + 1) * 1024])
                wo_sbs.append(w)

        # ---- phase 1: k^T (roped) and v_ext ----
        with tc.tile_pool(name="pk1", bufs=2, space="PSUM") as pk1, \
             tc.tile_pool(name="pv1", bufs=2, space="PSUM") as pv1, \
             tc.tile_pool(name="ktmp", bufs=2) as ktmp:
            for th in range(2):
                sl = slice(th * 512, (th + 1) * 512)
                pk = pk1.tile([128, 512], dt)
                for c in range(DC):
                    nc.tensor.matmul(pk, wk_sb[:, c, :], xT[:, c, sl],
                                     start=(c == 0), stop=(c == DC - 1))
                tmp = ktmp.tile([128, 512], dt)
                stage = ktmp.tile([128, 512], dt, tag="stage", name="kstage")
                _rope(nc, kT[:, sl], pk[:, :], cosk[:, sl], sink[:, sl], tmp,
                      stage)
            for tb in range(TT):
                pv = pv1.tile([128, 128], dt)
                for c in range(DC):
                    nc.tensor.matmul(pv, xT[:, c, tb * 128:(tb + 1) * 128],
                                     wv_sb[:, c, :],
                                     start=(c == 0), stop=(c == DC - 1))
                nc.scalar.copy(out=vext[:, tb, 0:128], in_=pv)
                nc.scalar.copy(out=vext[:, tb, 128:129], in_=onecol)

        # ---- phase 2: per-head q-proj + rope + causal attention ----
        with tc.tile_pool(name="qtp", bufs=2) as qtp, \
             tc.tile_pool(name="ropet", bufs=2) as ropet, \
             tc.tile_pool(name="expp", bufs=3) as expp, \
             tc.tile_pool(name="encp", bufs=3) as encp, \
             tc.tile_pool(name="recp", bufs=2) as recp, \
             tc.tile_pool(name="pq2", bufs=2, space="PSUM") as pq2, \
             tc.tile_pool(name="pl2", bufs=2, space="PSUM") as pl2, \
             tc.tile_pool(name="pe2", bufs=1, space="PSUM") as pe2, \
             tc.tile_pool(name="pt2", bufs=2, space="PSUM") as pt2:
            for n in range(NL):
                qT = qtp.tile([128, T], BF16)
                for th in range(2):
                    sl = slice(th * 512, (th + 1) * 512)
                    pq = pq2.tile([128, 512], dt)
                    for c in range(DC):
                        nc.tensor.matmul(pq, wq_sbs[n][:, c, :], xT[:, c, sl],
                                         start=(c == 0), stop=(c == DC - 1))
                    tmp = ropet.tile([128, 512], dt)
                    stage = ropet.tile([128, 512], dt, tag="qstage",
                                       name="qstage")
                    _rope(nc, qT[:, sl], pq, cosq[:, sl], sinq[:, sl], tmp,
                          stage)
                for qb in range(4):          # q blocks of 256 rows
                    R = qb * 256
                    d0 = R // 128            # diagonal chunk of sub0
                    d1 = d0 + 1              # diagonal chunk of sub1 (last)
                    pe0 = pe2.tile([128, EXPAD], dt, tag="pe0", name="pe0")
                    pe1 = pe2.tile([128, EXPAD], dt, tag="pe1", name="pe1")
                    for kc in range(d1 + 1):
                        plt = pl2.tile([128, 256], dt)
                        ex = expp.tile([128, 256], BF16)
                        if kc < d1:
                            nc.tensor.matmul(plt,
                                             kT[:, kc * 128:(kc + 1) * 128],
                                             qT[:, R:R + 256],
                                             start=True, stop=True)
                            nc.scalar.activation(
                                out=ex, in_=plt,
                                func=mybir.ActivationFunctionType.Exp)
                        else:
                            # sub0 is fully masked in its last chunk; compute
                            # only the sub1 half.
                            nc.tensor.matmul(plt[:, 128:256],
                                             kT[:, kc * 128:(kc + 1) * 128],
                                             qT[:, R + 128:R + 256],
                                             start=True, stop=True)
                            nc.scalar.activation(
                                out=ex[:, 128:256], in_=plt[:, 128:256],
                                func=mybir.ActivationFunctionType.Exp)
                        if kc == d0:
                            nc.vector.tensor_mul(ex[:, 0:128], ex[:, 0:128],
                                                 tri)
                        if kc == d1:
                            nc.vector.tensor_mul(ex[:, 128:256],
                                                 ex[:, 128:256], tri)
                        if kc <= d0:
                            nc.tensor.matmul(pe0, ex[:, 0:128],
                                             vext[:, kc, :],
                                             start=(kc == 0), stop=(kc == d0))
                        nc.tensor.matmul(pe1, ex[:, 128:256],
                                         vext[:, kc, :],
                                         start=(kc == 0), stop=(kc == d1))
                    for s, pes in ((0, pe0), (1, pe1)):
                        ts = d0 + s
                        rc = recp.tile([128, 1], dt)
                        nc.vector.reciprocal(rc, pes[:, 128:129])
                        en = encp.tile([128, 128], BF16)
                        nc.vector.tensor_scalar_mul(en, pes[:, 0:128], rc)
                        ptt = pt2.tile([128, 128], BF16)
                        nc.tensor.transpose(ptt, en, ident)
                        nc.vector.tensor_copy(out=encT[:, n, ts, :], in_=ptt)

        # ---- phase 3: output projection, accumulate over heads ----
        with tc.tile_pool(name="outp", bufs=4) as outp, \
             tc.tile_pool(name="po3", bufs=1, space="PSUM") as po3:
            for dh in range(2):
                for tg in range(2):
                    pos = [po3.tile([128, 1024], dt, tag=f"po{j}",
                                    name=f"po_t{j}") for j in range(4)]
                    for n in range(NL):
                        for j in range(4):
                            ts = tg * 4 + j
                            for c2 in range(2):
                                nc.tensor.matmul(
                                    pos[j][:, c2 * 512:(c2 + 1) * 512],
                                    encT[:, n, ts, :],
                                    wo_sbs[dh * NL + n][:,
                                                        c2 * 512:(c2 + 1) * 512],
                                    start=(n == 0), stop=(n == NL - 1))
                    for j in range(4):
                        ts = tg * 4 + j
                        ob = outp.tile([128, 1024], dt)
                        nc.vector.tensor_copy(out=ob, in_=pos[j])
                        nc.scalar.dma_start(
                            out=out_d[ts * 128:(ts + 1) * 128,
                                      dh * 1024:(dh + 1) * 1024],
                            in_=ob)
    nc.compile()
    return nc


def make_in_maps(x, wq, wkv, wo, segment_pos, attn_mask):
    x = np.asarray(x, dtype=np.float32)
    wq = np.asarray(wq, dtype=np.float32)
    wkv = np.asarray(wkv, dtype=np.float32)
    wo = np.asarray(wo, dtype=np.float32)
    segment_pos = np.asarray(segment_pos)
    attn_mask = np.asarray(attn_mask)

    # rope-pair interleave permutation (see SHUF_MASK): lane j of quadrant qd
    # holds orig dim qd*16+(j%16) for lanes 0-15, 64+qd*16+(j%16) for 16-31.
    lanes = np.arange(HD)
    qd, lane = lanes // 32, lanes % 32
    f = qd * 16 + (lane % 16)
    perm = np.where(lane < 16, f, HHD + f)
    sgn = np.where(lane < 16, np.float32(-1.0), np.float32(1.0))

    wk = np.ascontiguousarray(wkv[0, 0][:, perm].astype(NP_BF16))
    wv = np.ascontiguousarray(wkv[1, 0].astype(NP_BF16))
    frac = (2.0 / HD) * np.arange(HHD, dtype=np.float32)
    timescale = (np.float32(10000.0) ** frac).astype(np.float32)
    scale = np.float32(HD ** -0.5)

    in_maps = []
    for c in range(8):
        b, g = c // 2, c % 2
        pos = segment_pos[b].astype(np.float32)
        sinus = pos[:, None] / timescale[None, :]          # [T, 64]
        cos = np.cos(sinus).astype(np.float32).T           # [64, T]
        sin = np.sin(sinus).astype(np.float32).T
        cosD = cos[f, :]                                   # [128, T]
        sinS = sgn[:, None] * sin[f, :]
        tri = np.ascontiguousarray(
            attn_mask[b, :128, :128].T.astype(NP_BF16))    # 0/1: bf16-exact
        in_maps.append({
            "x": np.ascontiguousarray(x[b].astype(NP_BF16)),
            "wq": np.ascontiguousarray(
                wq[g * NL:(g + 1) * NL][:, :, perm].astype(NP_BF16)),
            "wk": wk,
            "wv": wv,
            "wo": np.ascontiguousarray(
                wo[g * NL:(g + 1) * NL].astype(NP_BF16)),
            "cosq": np.ascontiguousarray(cosD * scale),
            "sinq": np.ascontiguousarray(sinS * scale),
            "cosk": np.ascontiguousarray(cosD),
            "sink": np.ascontiguousarray(sinS),
            "tri": tri,
        })
    return in_maps


_NC_CACHE = None


def kernel(**inputs):
    global _NC_CACHE
    if _NC_CACHE is None:
        _NC_CACHE = build_nc()
    nc = _NC_CACHE
    in_maps = make_in_maps(
        inputs["x"], inputs["wq"], inputs["wkv"], inputs["wo"],
        inputs["segment_pos"], inputs["attn_mask"])
    res = run_bass_kernel_spmd(nc, in_maps, core_ids=list(range(8)))
    out = np.empty((B, T, D), dtype=np.float32)
    for b in range(B):
        out[b] = res.results[2 * b]["out"] + res.results[2 * b + 1]["out"]
    return out
